# revision 26
# baseline (speedup 1.0000x reference)
"""Slot-attention kernel for Trainium2, SPMD over 8 NeuronCores.

Reference computation (per batch element b):
  query[b,n,:] = q[n,b,:] @ qw[n]          (n = 32 query slots)
  keyp [b,m,:] = k[m,b,:] @ kw[m]          (m = 32 key slots)
  value[b,m,:] = k[m,b,:] @ vw[m]
  logits[b,n,m] = query[b,n,:]·keyp[b,m,:] / 16
  attn = softmax_m(logits)
  out[n,b,:] = sum_m attn[b,n,m] * value[b,m,:]

Sharding: data-parallel over batch (4096 -> 512 per core), weights replicated.
Host pre-casts to bf16 and pre-transposes q/k to [slot, dim, batch] so every
DMA is contiguous and the contraction dim (dim) lands on SBUF partitions.

Per-core schedule (two batch halves of 256):
  A) per-slot projections on PE (moving dim = batch), psum -> resident bf16
     slabs QTs/KTs/VTs; the 1/16 temperature is folded into the Q copy.
  B) per-4-batch-group logits via col-tiled matmuls (4 batches stacked on
     psum partitions), softmax over the free dim, normalization folded into
     E, then PE-transposes pack E^T for two groups per [64,128] tile.
  C) V shuffled (via idle GpSimd SWDGE) into [m, o, b] layout replicated on
     two partition row-blocks; attn@value runs as 8-way row+col tile-packed
     matmuls; plain psum->sbuf copies; 4 output DMAs per (half, o-quarter).
"""

import numpy as np
import ml_dtypes

import concourse.bass as bass
from concourse import bacc
import concourse.mybir as mybir
import concourse.tile as tile
from concourse.bass_utils import run_bass_kernel_spmd
from concourse.masks import make_identity
import concourse.bass_utils as _bu

# walrus defaults to --enable-ldw-opt=false, which forces every matmul to
# serialize behind its weight load; flip it so LDWEIGHTS can use the
# background weight buffer (validated by rel-err check in the harness).
if not getattr(_bu, "_ldw_opt_patched", False):
    _orig_run_command = _bu.run_command

    def _run_command_ldw(cmd, **kw):
        pass  # ldw-opt incompatible with our tile_position ldweights
        return _orig_run_command(cmd, **kw)

    _bu.run_command = _run_command_ldw
    _bu._ldw_opt_patched = True

BF16 = mybir.dt.bfloat16
F32 = mybir.dt.float32

NQ = 32          # query slots
NK = 32          # key slots
D = 256          # input dim (contraction of projections)
A = 256          # attn dim (contraction of logits)
O = 256          # out dim
BS = 4096
N_CORES = 8
BS_CORE = BS // N_CORES   # 512


def build_kernel(bs_core=BS_CORE, n_halves=2):
    """Builds the per-core Bass graph. bs_core must be divisible by 16*n_halves."""
    nc = bacc.Bacc()

    b_h = bs_core // n_halves          # batch per half (256)
    n_groups = b_h // 4                # 4-batch groups per half (64)
    n_gpairs = n_groups // 2

    qT = nc.declare_dram_parameter("qT", [NQ, D, bs_core], BF16, isOutput=False)
    kT = nc.declare_dram_parameter("kT", [NK, D, bs_core], BF16, isOutput=False)
    # merged per-slot weights: [slot, d, 3 (q/k/v), a]
    wall = nc.declare_dram_parameter("wall", [NQ, D, 3, A], BF16,
                                     isOutput=False)
    out = nc.declare_dram_parameter("out", [NQ, bs_core, O], F32, isOutput=True)

    SG = 2  # slots per input DMA group
    # [slot, d, b] -> partition = d%128, chunk c = d//128
    qT_g = qT.rearrange("(sg s) (c p) b -> sg p (s c) b", p=128, s=SG)
    kT_g = kT.rearrange("(sg s) (c p) b -> sg p (s c) b", p=128, s=SG)
    wall_g = wall.rearrange("(sg s) (c p) w a -> sg p (s c) (w a)", p=128, s=SG)

    with tile.TileContext(nc) as tc:
        with (
            tc.tile_pool(name="const", bufs=1) as const_pool,
            tc.tile_pool(name="win", bufs=2) as win,
            tc.tile_pool(name="xin", bufs=4) as xin,
            tc.tile_pool(name="big", bufs=1) as big,
            tc.tile_pool(name="outp", bufs=2) as outp,
            tc.tile_pool(name="vpool", bufs=1) as vpool,
            tc.tile_pool(name="etp", bufs=6) as etp,
            tc.tile_pool(name="proj_ps", bufs=2, space="PSUM") as proj_ps,
            tc.tile_pool(name="lg_ps", bufs=2, space="PSUM") as lg_ps,
            tc.tile_pool(name="tp_ps", bufs=2, space="PSUM") as tp_ps,
            tc.tile_pool(name="av_ps", bufs=2, space="PSUM") as av_ps,
        ):
            identity = const_pool.tile([128, 128], BF16)
            make_identity(nc, identity)

            for half in range(n_halves):
                b0 = half * b_h
                # ---- Phase A: projections -> QTs/KTs (a,b) + VN (b,o) ----
                QTs = big.tile([128, 2, NQ, b_h], BF16, tag="QTs")
                KTs = big.tile([128, 2, NK, b_h], BF16, tag="KTs")
                # value in [b, o] layout: partition = b%128, bc = b//128;
                # four sub-slabs (by m-range) so the V32Q shuffle can start
                # before the whole projection phase finishes
                n_bc = (b_h + 127) // 128
                bw = min(128, b_h)
                VN0 = big.tile([128, n_bc, NK // 4, O], BF16, tag="VN0")
                VN1 = big.tile([128, n_bc, NK // 4, O], BF16, tag="VN1")
                VN2 = big.tile([128, n_bc, NK // 4, O], BF16, tag="VN2")
                VN3 = big.tile([128, n_bc, NK // 4, O], BF16, tag="VN3")
                VNs = (VN0, VN1, VN2, VN3)

                for sg in range(NQ // SG):
                    qts = xin.tile([128, SG, 2, b_h], BF16, tag="qts")
                    nc.sync.dma_start(out=qts,
                                      in_=qT_g[sg, :, :, b0:b0 + b_h])
                    kts = xin.tile([128, SG, 2, b_h], BF16, tag="kts")
                    nc.sync.dma_start(out=kts,
                                      in_=kT_g[sg, :, :, b0:b0 + b_h])
                    wsg = win.tile([128, SG, 2, 3, A], BF16, tag="wsg")
                    nc.sync.dma_start(out=wsg, in_=wall_g[sg])

                    for si in range(SG):
                        s = sg * SG + si
                        # Q and K projections: out = [a, b] per slot
                        for pi in range(2):
                            xs = qts if pi == 0 else kts
                            dst = QTs if pi == 0 else KTs
                            for t in range(2):  # a-tile
                                ps = proj_ps.tile([128, b_h], F32, tag="ps")
                                for c in range(2):
                                    nc.tensor.matmul(
                                        ps,
                                        lhsT=wsg[:, si, c, pi,
                                                 t * 128:(t + 1) * 128],
                                        rhs=xs[:, si, c, :],
                                        start=(c == 0),
                                        stop=(c == 1),
                                    )
                                if pi == 0:
                                    if t == 0:
                                        nc.scalar.mul(dst[:, t, s, :], ps,
                                                      1.0 / 16.0)
                                    else:
                                        nc.vector.tensor_scalar_mul(
                                            out=dst[:, t, s, :], in0=ps,
                                            scalar1=1.0 / 16.0)
                                else:
                                    if t == 0:
                                        nc.scalar.copy(out=dst[:, t, s, :],
                                                       in_=ps)
                                    else:
                                        nc.vector.tensor_copy(
                                            out=dst[:, t, s, :], in_=ps)
                        # V projection transposed: stationary = k chunk,
                        # moving = vw -> psum [b_chunk, o]
                        for bc in range(n_bc):  # b-chunk of 128
                            ps = proj_ps.tile([128, O], F32, tag="ps")
                            for c in range(2):
                                nc.tensor.matmul(
                                    ps[:bw, :],
                                    lhsT=kts[:, si, c,
                                             bc * bw:(bc + 1) * bw],
                                    rhs=wsg[:, si, c, 2, :],
                                    start=(c == 0),
                                    stop=(c == 1),
                                )
                            if bc == 0:
                                nc.scalar.copy(
                                    out=VNs[s // 8][:bw, bc, s % 8, :],
                                    in_=ps[:bw, :])
                            else:
                                nc.vector.tensor_copy(
                                    out=VNs[s // 8][:bw, bc, s % 8, :],
                                    in_=ps[:bw, :])

                # ---- V shuffle: V32Q[32r+m, g, o] = value[b0+64r+g][m, o]
                V32Q = vpool.tile([128, n_groups, O], BF16, tag="V32Q")
                _dmae = (nc.gpsimd, nc.sync, nc.scalar)
                di = 0
                for m in range(NK):
                    for r in range(4):
                        row = 32 * r + m
                        b_lo = (r * n_groups) % 128
                        _dmae[di % 3].dma_start(
                            out=V32Q[row:row + 1, :, :],
                            in_=VNs[m // 8][b_lo:b_lo + n_groups,
                                            (r * n_groups) // 128, m % 8, :],
                        )
                        di += 1

                # ---- Phase B1: logits + exp + rowsum, batched by group-quad ----
                rs = big.tile([128, n_groups], F32, tag="rs")
                E = big.tile([128, n_groups, NK], BF16, tag="E")

                n_quads = n_groups // 4
                for gq in range(n_quads):
                    lg = lg_ps.tile([128, 4, NK], F32, tag="lg")
                    for qi in range(4):
                        g = 4 * gq + qi
                        for c in range(2):  # waves: 4 col groups concurrent
                            for j in range(4):
                                b = g + n_groups * j
                                nc.tensor.matmul(
                                    lg[32 * j:32 * (j + 1), qi, :],
                                    lhsT=QTs[:, c, :, b],
                                    rhs=KTs[:, c, :, b],
                                    start=(c == 0),
                                    stop=(c == 1),
                                    tile_position=(0, 32 * j),
                                    skip_group_check=True,
                                )
                    # softmax over m without max-subtraction: logits carry
                    # the 1/16 so |logit| <= ~2 and exp cannot overflow;
                    # normalization is folded into the output copy.
                    # accum_out gives the row-sum for free.
                    sm = big.tile([128, 4], F32, tag="sm")
                    for qi in range(4):
                        nc.scalar.activation(
                            out=E[:, 4 * gq + qi, :], in_=lg[:, qi, :],
                            func=mybir.ActivationFunctionType.Exp,
                            accum_out=sm[:, qi:qi + 1],
                        )
                    nc.vector.reciprocal(out=rs[:, 4 * gq:4 * gq + 4], in_=sm)

                g_chunk = min(8, n_groups)
                for g0 in range(0, n_groups, g_chunk):
                    OUTo = outp.tile([128, g_chunk, O], F32, tag="OUTo")
                    for g in range(g0, g0 + g_chunk):
                        et = tp_ps.tile([128, 128], F32, tag="tp")
                        for j in range(4):
                            # E_block.T @ I-slice: batch (g + 64j)'s [m, n]
                            # onto the diagonal block, zeros elsewhere
                            nc.tensor.matmul(
                                et[32 * j:32 * (j + 1), :],
                                lhsT=E[32 * j:32 * (j + 1), g, :],
                                rhs=identity[32 * j:32 * (j + 1), :],
                                start=True, stop=True,
                                tile_position=(32 * j, 32 * j),
                                skip_group_check=True,
                            )
                        etb = etp.tile([128, 128], BF16, tag="etb")
                        if g % 2 == 0:
                            nc.scalar.copy(out=etb, in_=et)
                        else:
                            nc.vector.tensor_copy(out=etb, in_=et)
                        av = av_ps.tile([128, O], F32, tag="av")
                        nc.tensor.matmul(
                            av,
                            lhsT=etb,
                            rhs=V32Q[:, g, :],
                            start=True, stop=True,
                            skip_group_check=True,
                        )
                        # psum -> sbuf with 1/softmax-sum scaling per row
                        if g % 2 == 0:
                            nc.scalar.mul(OUTo[:, g - g0, :], av,
                                          rs[:, g:g + 1])
                        else:
                            nc.vector.tensor_scalar_mul(
                                out=OUTo[:, g - g0, :], in0=av,
                                scalar1=rs[:, g:g + 1])
                    # flush: 4 DMAs, one per batch stripe j (batches
                    # b0 + 64j + g0 .. +g_chunk are consecutive)
                    for j in range(4):
                        nc.sync.dma_start(
                            out=out[:, b0 + n_groups * j + g0:
                                    b0 + n_groups * j + g0 + g_chunk, :],
                            in_=OUTo[32 * j:32 * (j + 1), :, :],
                        )
    return nc


def _prep_inputs(q, k, query_weight, key_weight, value_weight, bs_core):
    bf = ml_dtypes.bfloat16
    wall = np.ascontiguousarray(
        np.stack((query_weight, key_weight, value_weight), axis=2)
    ).astype(bf)  # [slot, d, 3, a]
    in_maps = []
    for i in range(N_CORES):
        sl = slice(i * bs_core, (i + 1) * bs_core)
        qTb = np.ascontiguousarray(q[:, sl, :].transpose(0, 2, 1)).astype(bf)
        kTb = np.ascontiguousarray(k[:, sl, :].transpose(0, 2, 1)).astype(bf)
        in_maps.append({"qT": qTb, "kT": kTb, "wall": wall})
    return in_maps


_NC_CACHE = {}


def _get_nc(bs_core, n_halves=2):
    key = (bs_core, n_halves)
    if key not in _NC_CACHE:
        nc = build_kernel(bs_core, n_halves)
        nc.finalize()
        _NC_CACHE[key] = nc
    return _NC_CACHE[key]


def kernel(q, k, query_weight, key_weight, value_weight, _trace=False):
    nc = _get_nc(BS_CORE)
    in_maps = _prep_inputs(q, k, query_weight, key_weight, value_weight, BS_CORE)
    res = run_bass_kernel_spmd(nc, in_maps, core_ids=list(range(N_CORES)),
                               trace=_trace)
    outs = [res.results[i]["out"] for i in range(N_CORES)]
    full = np.concatenate(outs, axis=1).astype(np.float32)
    if _trace:
        return full, res
    return full


# revision 27
# speedup vs baseline: 1.1338x; 1.1338x over previous
"""Slot-attention kernel for Trainium2, SPMD over 8 NeuronCores.

Reference computation (per batch element b):
  query[b,n,:] = q[n,b,:] @ qw[n]          (n = 32 query slots)
  keyp [b,m,:] = k[m,b,:] @ kw[m]          (m = 32 key slots)
  value[b,m,:] = k[m,b,:] @ vw[m]
  logits[b,n,m] = query[b,n,:]·keyp[b,m,:] / 16
  attn = softmax_m(logits)
  out[n,b,:] = sum_m attn[b,n,m] * value[b,m,:]

Sharding: data-parallel over batch (4096 -> 512 per core), weights replicated.
Host pre-casts to bf16 and pre-transposes q/k to [slot, dim, batch] so every
DMA is contiguous and the contraction dim (dim) lands on SBUF partitions.

Per-core schedule (two batch halves of 256):
  A) per-slot projections on PE (moving dim = batch), psum -> resident bf16
     slabs QTs/KTs/VTs; the 1/16 temperature is folded into the Q copy.
  B) per-4-batch-group logits via col-tiled matmuls (4 batches stacked on
     psum partitions), softmax over the free dim, normalization folded into
     E, then PE-transposes pack E^T for two groups per [64,128] tile.
  C) V shuffled (via idle GpSimd SWDGE) into [m, o, b] layout replicated on
     two partition row-blocks; attn@value runs as 8-way row+col tile-packed
     matmuls; plain psum->sbuf copies; 4 output DMAs per (half, o-quarter).
"""

import numpy as np
import ml_dtypes

import concourse.bass as bass
from concourse import bacc
import concourse.mybir as mybir
import concourse.tile as tile
from concourse.bass_utils import run_bass_kernel_spmd
from concourse.masks import make_identity
import concourse.bass_utils as _bu

# walrus defaults to --enable-ldw-opt=false, which forces every matmul to
# serialize behind its weight load; flip it so LDWEIGHTS can use the
# background weight buffer (validated by rel-err check in the harness).
if not getattr(_bu, "_ldw_opt_patched", False):
    _orig_run_command = _bu.run_command

    def _run_command_ldw(cmd, **kw):
        pass  # ldw-opt incompatible with our tile_position ldweights
        return _orig_run_command(cmd, **kw)

    _bu.run_command = _run_command_ldw
    _bu._ldw_opt_patched = True

BF16 = mybir.dt.bfloat16
F32 = mybir.dt.float32

NQ = 32          # query slots
NK = 32          # key slots
D = 256          # input dim (contraction of projections)
A = 256          # attn dim (contraction of logits)
O = 256          # out dim
BS = 4096
N_CORES = 8
BS_CORE = BS // N_CORES   # 512


def build_kernel(bs_core=BS_CORE, n_halves=2):
    """Builds the per-core Bass graph. bs_core must be divisible by 16*n_halves."""
    nc = bacc.Bacc()

    b_h = bs_core // n_halves          # batch per half (256)
    n_groups = b_h // 4                # 4-batch groups per half (64)
    n_gpairs = n_groups // 2

    qT = nc.declare_dram_parameter("qT", [NQ, D, bs_core], BF16, isOutput=False)
    kT = nc.declare_dram_parameter("kT", [NK, D, bs_core], BF16, isOutput=False)
    # merged per-slot weights: [slot, d, 3 (q/k/v), a]
    wall = nc.declare_dram_parameter("wall", [NQ, D, 3, A], BF16,
                                     isOutput=False)
    out = nc.declare_dram_parameter("out", [NQ, bs_core, O], F32, isOutput=True)

    SG = 2  # slots per input DMA group
    # [slot, d, b] -> partition = d%128, chunk c = d//128
    qT_g = qT.rearrange("(sg s) (c p) b -> sg p (s c) b", p=128, s=SG)
    kT_g = kT.rearrange("(sg s) (c p) b -> sg p (s c) b", p=128, s=SG)
    wall_g = wall.rearrange("(sg s) (c p) w a -> sg p (s c) (w a)", p=128, s=SG)

    with tile.TileContext(nc) as tc:
        with (
            tc.tile_pool(name="const", bufs=1) as const_pool,
            tc.tile_pool(name="win", bufs=2) as win,
            tc.tile_pool(name="xin", bufs=4) as xin,
            tc.tile_pool(name="big", bufs=1) as big,
            tc.tile_pool(name="outp", bufs=2) as outp,
            tc.tile_pool(name="vpool", bufs=1) as vpool,
            tc.tile_pool(name="etp", bufs=6) as etp,
            tc.tile_pool(name="proj_ps", bufs=2, space="PSUM") as proj_ps,
            tc.tile_pool(name="lg_ps", bufs=2, space="PSUM") as lg_ps,
            tc.tile_pool(name="tp_ps", bufs=2, space="PSUM") as tp_ps,
            tc.tile_pool(name="av_ps", bufs=2, space="PSUM") as av_ps,
        ):
            identity = const_pool.tile([128, 128], BF16)
            make_identity(nc, identity)

            for half in range(n_halves):
                b0 = half * b_h
                # ---- Phase A: projections -> QTs/KTs (a,b) + VN (b,o) ----
                QTs = big.tile([128, 2, NQ, b_h], BF16, tag="QTs")
                KTs = big.tile([128, 2, NK, b_h], BF16, tag="KTs")
                # value in [b, o] layout: partition = b%128, bc = b//128;
                # four sub-slabs (by m-range) so the V32Q shuffle can start
                # before the whole projection phase finishes
                n_bc = (b_h + 127) // 128
                bw = min(128, b_h)
                VN0 = big.tile([128, n_bc, NK // 4, O], BF16, tag="VN0")
                VN1 = big.tile([128, n_bc, NK // 4, O], BF16, tag="VN1")
                VN2 = big.tile([128, n_bc, NK // 4, O], BF16, tag="VN2")
                VN3 = big.tile([128, n_bc, NK // 4, O], BF16, tag="VN3")
                VNs = (VN0, VN1, VN2, VN3)

                for sg in range(NQ // SG):
                    qts = xin.tile([128, SG, 2, b_h], BF16, tag="qts")
                    nc.sync.dma_start(out=qts,
                                      in_=qT_g[sg, :, :, b0:b0 + b_h])
                    kts = xin.tile([128, SG, 2, b_h], BF16, tag="kts")
                    nc.sync.dma_start(out=kts,
                                      in_=kT_g[sg, :, :, b0:b0 + b_h])
                    wsg = win.tile([128, SG, 2, 3, A], BF16, tag="wsg")
                    nc.sync.dma_start(out=wsg, in_=wall_g[sg])

                    for si in range(SG):
                        s = sg * SG + si
                        # Q and K projections: out = [a, b] per slot
                        for pi in range(2):
                            xs = qts if pi == 0 else kts
                            dst = QTs if pi == 0 else KTs
                            for t in range(2):  # a-tile
                                ps = proj_ps.tile([128, b_h], F32, tag="ps")
                                for c in range(2):
                                    nc.tensor.matmul(
                                        ps,
                                        lhsT=wsg[:, si, c, pi,
                                                 t * 128:(t + 1) * 128],
                                        rhs=xs[:, si, c, :],
                                        start=(c == 0),
                                        stop=(c == 1),
                                    )
                                if pi == 0:
                                    if t == 0:
                                        nc.scalar.mul(dst[:, t, s, :], ps,
                                                      1.0 / 16.0)
                                    else:
                                        nc.vector.tensor_scalar_mul(
                                            out=dst[:, t, s, :], in0=ps,
                                            scalar1=1.0 / 16.0)
                                else:
                                    if t == 0:
                                        nc.scalar.copy(out=dst[:, t, s, :],
                                                       in_=ps)
                                    else:
                                        nc.vector.tensor_copy(
                                            out=dst[:, t, s, :], in_=ps)
                        # V projection transposed: stationary = k chunk,
                        # moving = vw -> psum [b_chunk, o]
                        for bc in range(n_bc):  # b-chunk of 128
                            ps = proj_ps.tile([128, O], F32, tag="ps")
                            for c in range(2):
                                nc.tensor.matmul(
                                    ps[:bw, :],
                                    lhsT=kts[:, si, c,
                                             bc * bw:(bc + 1) * bw],
                                    rhs=wsg[:, si, c, 2, :],
                                    start=(c == 0),
                                    stop=(c == 1),
                                )
                            if bc == 0:
                                nc.scalar.copy(
                                    out=VNs[s // 8][:bw, bc, s % 8, :],
                                    in_=ps[:bw, :])
                            else:
                                nc.vector.tensor_copy(
                                    out=VNs[s // 8][:bw, bc, s % 8, :],
                                    in_=ps[:bw, :])

                # ---- V shuffle: V32Q[32r+m, g, o] = value[b0+64r+g][m, o]
                V32Q = vpool.tile([128, n_groups, O], BF16, tag="V32Q")
                _dmae = (nc.gpsimd, nc.sync)
                di = 0
                for m in range(NK):
                    for r in range(4):
                        row = 32 * r + m
                        b_lo = (r * n_groups) % 128
                        _dmae[di % 2].dma_start(
                            out=V32Q[row:row + 1, :, :],
                            in_=VNs[m // 8][b_lo:b_lo + n_groups,
                                            (r * n_groups) // 128, m % 8, :],
                        )
                        di += 1

                # ---- Phase B1: logits + exp + rowsum, batched by group-quad ----
                rs = big.tile([128, n_groups], F32, tag="rs")
                E = big.tile([128, n_groups, NK], BF16, tag="E")

                n_quads = n_groups // 4
                for gq in range(n_quads):
                    lg = lg_ps.tile([128, 4, NK], F32, tag="lg")
                    for qi in range(4):
                        g = 4 * gq + qi
                        for c in range(2):  # waves: 4 col groups concurrent
                            for j in range(4):
                                b = g + n_groups * j
                                nc.tensor.matmul(
                                    lg[32 * j:32 * (j + 1), qi, :],
                                    lhsT=QTs[:, c, :, b],
                                    rhs=KTs[:, c, :, b],
                                    start=(c == 0),
                                    stop=(c == 1),
                                    tile_position=(0, 32 * j),
                                    skip_group_check=True,
                                )
                    # softmax over m without max-subtraction: logits carry
                    # the 1/16 so |logit| <= ~2 and exp cannot overflow;
                    # normalization is folded into the output copy.
                    # accum_out gives the row-sum for free.
                    sm = big.tile([128, 4], F32, tag="sm")
                    for qi in range(4):
                        nc.scalar.activation(
                            out=E[:, 4 * gq + qi, :], in_=lg[:, qi, :],
                            func=mybir.ActivationFunctionType.Exp,
                            accum_out=sm[:, qi:qi + 1],
                        )
                    nc.vector.reciprocal(out=rs[:, 4 * gq:4 * gq + 4], in_=sm)

                g_chunk = min(8, n_groups)
                for g0 in range(0, n_groups, g_chunk):
                    OUTo = outp.tile([128, g_chunk, O], F32, tag="OUTo")
                    for g in range(g0, g0 + g_chunk):
                        et = tp_ps.tile([128, 128], F32, tag="tp")
                        for j in range(4):
                            # E_block.T @ I-slice: batch (g + 64j)'s [m, n]
                            # onto the diagonal block, zeros elsewhere
                            nc.tensor.matmul(
                                et[32 * j:32 * (j + 1), :],
                                lhsT=E[32 * j:32 * (j + 1), g, :],
                                rhs=identity[32 * j:32 * (j + 1), :],
                                start=True, stop=True,
                                tile_position=(32 * j, 32 * j),
                                skip_group_check=True,
                            )
                        etb = etp.tile([128, 128], BF16, tag="etb")
                        if g % 2 == 0:
                            nc.scalar.copy(out=etb, in_=et)
                        else:
                            nc.vector.tensor_copy(out=etb, in_=et)
                        av = av_ps.tile([128, O], F32, tag="av")
                        nc.tensor.matmul(
                            av,
                            lhsT=etb,
                            rhs=V32Q[:, g, :],
                            start=True, stop=True,
                            skip_group_check=True,
                        )
                        # psum -> sbuf with 1/softmax-sum scaling per row
                        if g % 2 == 0:
                            nc.scalar.mul(OUTo[:, g - g0, :], av,
                                          rs[:, g:g + 1])
                        else:
                            nc.vector.tensor_scalar_mul(
                                out=OUTo[:, g - g0, :], in0=av,
                                scalar1=rs[:, g:g + 1])
                    # flush: 4 DMAs, one per batch stripe j (batches
                    # b0 + 64j + g0 .. +g_chunk are consecutive)
                    for j in range(4):
                        nc.sync.dma_start(
                            out=out[:, b0 + n_groups * j + g0:
                                    b0 + n_groups * j + g0 + g_chunk, :],
                            in_=OUTo[32 * j:32 * (j + 1), :, :],
                        )
    return nc


def _prep_inputs(q, k, query_weight, key_weight, value_weight, bs_core):
    bf = ml_dtypes.bfloat16
    wall = np.ascontiguousarray(
        np.stack((query_weight, key_weight, value_weight), axis=2)
    ).astype(bf)  # [slot, d, 3, a]
    in_maps = []
    for i in range(N_CORES):
        sl = slice(i * bs_core, (i + 1) * bs_core)
        qTb = np.ascontiguousarray(q[:, sl, :].transpose(0, 2, 1)).astype(bf)
        kTb = np.ascontiguousarray(k[:, sl, :].transpose(0, 2, 1)).astype(bf)
        in_maps.append({"qT": qTb, "kT": kTb, "wall": wall})
    return in_maps


_NC_CACHE = {}


def _get_nc(bs_core, n_halves=2):
    key = (bs_core, n_halves)
    if key not in _NC_CACHE:
        nc = build_kernel(bs_core, n_halves)
        nc.finalize()
        _NC_CACHE[key] = nc
    return _NC_CACHE[key]


def kernel(q, k, query_weight, key_weight, value_weight, _trace=False):
    nc = _get_nc(BS_CORE)
    in_maps = _prep_inputs(q, k, query_weight, key_weight, value_weight, BS_CORE)
    res = run_bass_kernel_spmd(nc, in_maps, core_ids=list(range(N_CORES)),
                               trace=_trace)
    outs = [res.results[i]["out"] for i in range(N_CORES)]
    full = np.concatenate(outs, axis=1).astype(np.float32)
    if _trace:
        return full, res
    return full


# revision 28
# speedup vs baseline: 1.1542x; 1.0180x over previous
"""Slot-attention kernel for Trainium2, SPMD over 8 NeuronCores.

Reference computation (per batch element b):
  query[b,n,:] = q[n,b,:] @ qw[n]          (n = 32 query slots)
  keyp [b,m,:] = k[m,b,:] @ kw[m]          (m = 32 key slots)
  value[b,m,:] = k[m,b,:] @ vw[m]
  logits[b,n,m] = query[b,n,:]·keyp[b,m,:] / 16
  attn = softmax_m(logits)
  out[n,b,:] = sum_m attn[b,n,m] * value[b,m,:]

Sharding: data-parallel over batch (4096 -> 512 per core), weights replicated.
Host pre-casts to bf16 and pre-transposes q/k to [slot, dim, batch] so every
DMA is contiguous and the contraction dim (dim) lands on SBUF partitions.

Per-core schedule (two batch halves of 256):
  A) per-slot projections on PE (moving dim = batch), psum -> resident bf16
     slabs QTs/KTs/VTs; the 1/16 temperature is folded into the Q copy.
  B) per-4-batch-group logits via col-tiled matmuls (4 batches stacked on
     psum partitions), softmax over the free dim, normalization folded into
     E, then PE-transposes pack E^T for two groups per [64,128] tile.
  C) V shuffled (via idle GpSimd SWDGE) into [m, o, b] layout replicated on
     two partition row-blocks; attn@value runs as 8-way row+col tile-packed
     matmuls; plain psum->sbuf copies; 4 output DMAs per (half, o-quarter).
"""

import numpy as np
import ml_dtypes

import concourse.bass as bass
from concourse import bacc
import concourse.mybir as mybir
import concourse.tile as tile
from concourse.bass_utils import run_bass_kernel_spmd
from concourse.masks import make_identity
import concourse.bass_utils as _bu

# walrus defaults to --enable-ldw-opt=false, which forces every matmul to
# serialize behind its weight load; flip it so LDWEIGHTS can use the
# background weight buffer (validated by rel-err check in the harness).
if not getattr(_bu, "_ldw_opt_patched", False):
    _orig_run_command = _bu.run_command

    def _run_command_ldw(cmd, **kw):
        pass  # ldw-opt incompatible with our tile_position ldweights
        return _orig_run_command(cmd, **kw)

    _bu.run_command = _run_command_ldw
    _bu._ldw_opt_patched = True

BF16 = mybir.dt.bfloat16
F32 = mybir.dt.float32

NQ = 32          # query slots
NK = 32          # key slots
D = 256          # input dim (contraction of projections)
A = 256          # attn dim (contraction of logits)
O = 256          # out dim
BS = 4096
N_CORES = 8
BS_CORE = BS // N_CORES   # 512


def build_kernel(bs_core=BS_CORE, n_halves=2):
    """Builds the per-core Bass graph. bs_core must be divisible by 16*n_halves."""
    nc = bacc.Bacc()

    b_h = bs_core // n_halves          # batch per half (256)
    n_groups = b_h // 4                # 4-batch groups per half (64)
    n_gpairs = n_groups // 2

    qT = nc.declare_dram_parameter("qT", [NQ, D, bs_core], BF16, isOutput=False)
    kT = nc.declare_dram_parameter("kT", [NK, D, bs_core], BF16, isOutput=False)
    # merged per-slot weights: [slot, d, 3 (q/k/v), a]
    wall = nc.declare_dram_parameter("wall", [NQ, D, 3, A], BF16,
                                     isOutput=False)
    out = nc.declare_dram_parameter("out", [NQ, bs_core, O], F32, isOutput=True)

    SG = 2  # slots per input DMA group
    # [slot, d, b] -> partition = d%128, chunk c = d//128
    qT_g = qT.rearrange("(sg s) (c p) b -> sg p (s c) b", p=128, s=SG)
    kT_g = kT.rearrange("(sg s) (c p) b -> sg p (s c) b", p=128, s=SG)
    wall_g = wall.rearrange("(sg s) (c p) w a -> sg p (s c) (w a)", p=128, s=SG)

    with tile.TileContext(nc) as tc:
        with (
            tc.tile_pool(name="const", bufs=1) as const_pool,
            tc.tile_pool(name="win", bufs=2) as win,
            tc.tile_pool(name="xin", bufs=4) as xin,
            tc.tile_pool(name="big", bufs=1) as big,
            tc.tile_pool(name="outp", bufs=2) as outp,
            tc.tile_pool(name="vpool", bufs=1) as vpool,
            tc.tile_pool(name="etp", bufs=6) as etp,
            tc.tile_pool(name="proj_ps", bufs=2, space="PSUM") as proj_ps,
            tc.tile_pool(name="lg_ps", bufs=2, space="PSUM") as lg_ps,
            tc.tile_pool(name="tp_ps", bufs=2, space="PSUM") as tp_ps,
            tc.tile_pool(name="av_ps", bufs=2, space="PSUM") as av_ps,
        ):
            identity = const_pool.tile([128, 128], BF16)
            make_identity(nc, identity)

            for half in range(n_halves):
                b0 = half * b_h
                # ---- Phase A: projections -> QTs/KTs (a,b) + VN (b,o) ----
                QTs = big.tile([128, 2, NQ, b_h], BF16, tag="QTs")
                KTs = big.tile([128, 2, NK, b_h], BF16, tag="KTs")
                # value in [b, o] layout: partition = b%128, bc = b//128;
                # four sub-slabs (by m-range) so the V32Q shuffle can start
                # before the whole projection phase finishes
                n_bc = (b_h + 127) // 128
                bw = min(128, b_h)
                VN0 = big.tile([128, n_bc, NK // 4, O], BF16, tag="VN0")
                VN1 = big.tile([128, n_bc, NK // 4, O], BF16, tag="VN1")
                VN2 = big.tile([128, n_bc, NK // 4, O], BF16, tag="VN2")
                VN3 = big.tile([128, n_bc, NK // 4, O], BF16, tag="VN3")
                VNs = (VN0, VN1, VN2, VN3)

                for sg in range(NQ // SG):
                    qts = xin.tile([128, SG, 2, b_h], BF16, tag="qts")
                    nc.sync.dma_start(out=qts,
                                      in_=qT_g[sg, :, :, b0:b0 + b_h])
                    kts = xin.tile([128, SG, 2, b_h], BF16, tag="kts")
                    nc.sync.dma_start(out=kts,
                                      in_=kT_g[sg, :, :, b0:b0 + b_h])
                    wsg = win.tile([128, SG, 2, 3, A], BF16, tag="wsg")
                    nc.sync.dma_start(out=wsg, in_=wall_g[sg])

                    for si in range(SG):
                        s = sg * SG + si
                        # Q and K projections: out = [a, b] per slot
                        for pi in range(2):
                            xs = qts if pi == 0 else kts
                            dst = QTs if pi == 0 else KTs
                            for t in range(2):  # a-tile
                                ps = proj_ps.tile([128, b_h], F32, tag="ps")
                                for c in range(2):
                                    nc.tensor.matmul(
                                        ps,
                                        lhsT=wsg[:, si, c, pi,
                                                 t * 128:(t + 1) * 128],
                                        rhs=xs[:, si, c, :],
                                        start=(c == 0),
                                        stop=(c == 1),
                                    )
                                if pi == 0:
                                    if t == 0:
                                        nc.scalar.mul(dst[:, t, s, :], ps,
                                                      1.0 / 16.0)
                                    else:
                                        nc.vector.tensor_scalar_mul(
                                            out=dst[:, t, s, :], in0=ps,
                                            scalar1=1.0 / 16.0)
                                else:
                                    if t == 0:
                                        nc.scalar.copy(out=dst[:, t, s, :],
                                                       in_=ps)
                                    else:
                                        nc.vector.tensor_copy(
                                            out=dst[:, t, s, :], in_=ps)
                        # V projection transposed: stationary = k chunk,
                        # moving = vw -> psum [b_chunk, o]
                        for bc in range(n_bc):  # b-chunk of 128
                            ps = proj_ps.tile([128, O], F32, tag="ps")
                            for c in range(2):
                                nc.tensor.matmul(
                                    ps[:bw, :],
                                    lhsT=kts[:, si, c,
                                             bc * bw:(bc + 1) * bw],
                                    rhs=wsg[:, si, c, 2, :],
                                    start=(c == 0),
                                    stop=(c == 1),
                                )
                            if bc == 0:
                                nc.scalar.copy(
                                    out=VNs[s // 8][:bw, bc, s % 8, :],
                                    in_=ps[:bw, :])
                            else:
                                nc.vector.tensor_copy(
                                    out=VNs[s // 8][:bw, bc, s % 8, :],
                                    in_=ps[:bw, :])

                # ---- V shuffle: V32Q[32r+m, g, o] = value[b0+64r+g][m, o]
                V32Q = vpool.tile([128, n_groups, O], BF16, tag="V32Q")
                _dmae = (nc.gpsimd, nc.sync)
                di = 0
                for m in range(NK):
                    for r in range(4):
                        row = 32 * r + m
                        b_lo = (r * n_groups) % 128
                        _dmae[di % 2].dma_start(
                            out=V32Q[row:row + 1, :, :],
                            in_=VNs[m // 8][b_lo:b_lo + n_groups,
                                            (r * n_groups) // 128, m % 8, :],
                        )
                        di += 1

                # ---- Phase B1: logits + exp + rowsum, batched by group-quad ----
                rs = big.tile([128, n_groups], F32, tag="rs")
                E = big.tile([128, n_groups, NK], BF16, tag="E")

                n_quads = n_groups // 4
                for gq in range(n_quads):
                    lg = lg_ps.tile([128, 4, NK], F32, tag="lg")
                    for qi in range(4):
                        g = 4 * gq + qi
                        for c in range(2):  # waves: 4 col groups concurrent
                            for j in range(4):
                                b = g + n_groups * j
                                nc.tensor.matmul(
                                    lg[32 * j:32 * (j + 1), qi, :],
                                    lhsT=QTs[:, c, :, b],
                                    rhs=KTs[:, c, :, b],
                                    start=(c == 0),
                                    stop=(c == 1),
                                    tile_position=(0, 32 * j),
                                    skip_group_check=True,
                                )
                    # softmax over m without max-subtraction: logits carry
                    # the 1/16 so |logit| <= ~2 and exp cannot overflow;
                    # normalization is folded into the output copy
                    sm = big.tile([128, 4], F32, tag="sm")
                    nc.scalar.activation(
                        out=E[:, 4 * gq:4 * gq + 4, :], in_=lg,
                        func=mybir.ActivationFunctionType.Exp,
                    )
                    nc.vector.reduce_sum(
                        out=sm, in_=E[:, 4 * gq:4 * gq + 4, :],
                        axis=mybir.AxisListType.X,
                    )
                    nc.vector.reciprocal(out=rs[:, 4 * gq:4 * gq + 4], in_=sm)

                g_chunk = min(8, n_groups)
                for g0 in range(0, n_groups, g_chunk):
                    OUTo = outp.tile([128, g_chunk, O], F32, tag="OUTo")
                    for g in range(g0, g0 + g_chunk):
                        et = tp_ps.tile([128, 128], F32, tag="tp")
                        for j in range(4):
                            # E_block.T @ I-slice: batch (g + 64j)'s [m, n]
                            # onto the diagonal block, zeros elsewhere
                            nc.tensor.matmul(
                                et[32 * j:32 * (j + 1), :],
                                lhsT=E[32 * j:32 * (j + 1), g, :],
                                rhs=identity[32 * j:32 * (j + 1), :],
                                start=True, stop=True,
                                tile_position=(32 * j, 32 * j),
                                skip_group_check=True,
                            )
                        etb = etp.tile([128, 128], BF16, tag="etb")
                        if g % 2 == 0:
                            nc.scalar.copy(out=etb, in_=et)
                        else:
                            nc.vector.tensor_copy(out=etb, in_=et)
                        av = av_ps.tile([128, O], F32, tag="av")
                        nc.tensor.matmul(
                            av,
                            lhsT=etb,
                            rhs=V32Q[:, g, :],
                            start=True, stop=True,
                            skip_group_check=True,
                        )
                        # psum -> sbuf with 1/softmax-sum scaling per row
                        if g % 2 == 0:
                            nc.scalar.mul(OUTo[:, g - g0, :], av,
                                          rs[:, g:g + 1])
                        else:
                            nc.vector.tensor_scalar_mul(
                                out=OUTo[:, g - g0, :], in0=av,
                                scalar1=rs[:, g:g + 1])
                    # flush: 4 DMAs, one per batch stripe j (batches
                    # b0 + 64j + g0 .. +g_chunk are consecutive)
                    for j in range(4):
                        nc.sync.dma_start(
                            out=out[:, b0 + n_groups * j + g0:
                                    b0 + n_groups * j + g0 + g_chunk, :],
                            in_=OUTo[32 * j:32 * (j + 1), :, :],
                        )
    return nc


def _prep_inputs(q, k, query_weight, key_weight, value_weight, bs_core):
    bf = ml_dtypes.bfloat16
    wall = np.ascontiguousarray(
        np.stack((query_weight, key_weight, value_weight), axis=2)
    ).astype(bf)  # [slot, d, 3, a]
    in_maps = []
    for i in range(N_CORES):
        sl = slice(i * bs_core, (i + 1) * bs_core)
        qTb = np.ascontiguousarray(q[:, sl, :].transpose(0, 2, 1)).astype(bf)
        kTb = np.ascontiguousarray(k[:, sl, :].transpose(0, 2, 1)).astype(bf)
        in_maps.append({"qT": qTb, "kT": kTb, "wall": wall})
    return in_maps


_NC_CACHE = {}


def _get_nc(bs_core, n_halves=2):
    key = (bs_core, n_halves)
    if key not in _NC_CACHE:
        nc = build_kernel(bs_core, n_halves)
        nc.finalize()
        _NC_CACHE[key] = nc
    return _NC_CACHE[key]


def kernel(q, k, query_weight, key_weight, value_weight, _trace=False):
    nc = _get_nc(BS_CORE)
    in_maps = _prep_inputs(q, k, query_weight, key_weight, value_weight, BS_CORE)
    res = run_bass_kernel_spmd(nc, in_maps, core_ids=list(range(N_CORES)),
                               trace=_trace)
    outs = [res.results[i]["out"] for i in range(N_CORES)]
    full = np.concatenate(outs, axis=1).astype(np.float32)
    if _trace:
        return full, res
    return full


# revision 29
# speedup vs baseline: 1.2193x; 1.0564x over previous
"""Slot-attention kernel for Trainium2, SPMD over 8 NeuronCores.

Reference computation (per batch element b):
  query[b,n,:] = q[n,b,:] @ qw[n]          (n = 32 query slots)
  keyp [b,m,:] = k[m,b,:] @ kw[m]          (m = 32 key slots)
  value[b,m,:] = k[m,b,:] @ vw[m]
  logits[b,n,m] = query[b,n,:]·keyp[b,m,:] / 16
  attn = softmax_m(logits)
  out[n,b,:] = sum_m attn[b,n,m] * value[b,m,:]

Sharding: data-parallel over batch (4096 -> 512 per core), weights replicated.
Host pre-casts to bf16 and pre-transposes q/k to [slot, dim, batch] so every
DMA is contiguous and the contraction dim (dim) lands on SBUF partitions.

Per-core schedule (two batch halves of 256):
  A) per-slot projections on PE (moving dim = batch), psum -> resident bf16
     slabs QTs/KTs/VTs; the 1/16 temperature is folded into the Q copy.
  B) per-4-batch-group logits via col-tiled matmuls (4 batches stacked on
     psum partitions), softmax over the free dim, normalization folded into
     E, then PE-transposes pack E^T for two groups per [64,128] tile.
  C) V shuffled (via idle GpSimd SWDGE) into [m, o, b] layout replicated on
     two partition row-blocks; attn@value runs as 8-way row+col tile-packed
     matmuls; plain psum->sbuf copies; 4 output DMAs per (half, o-quarter).
"""

import numpy as np
import ml_dtypes

import concourse.bass as bass
from concourse import bacc
import concourse.mybir as mybir
import concourse.tile as tile
from concourse.bass_utils import run_bass_kernel_spmd
from concourse.masks import make_identity
import concourse.bass_utils as _bu

# walrus defaults to --enable-ldw-opt=false, which forces every matmul to
# serialize behind its weight load; flip it so LDWEIGHTS can use the
# background weight buffer (validated by rel-err check in the harness).
if not getattr(_bu, "_ldw_opt_patched", False):
    _orig_run_command = _bu.run_command

    def _run_command_ldw(cmd, **kw):
        pass  # ldw-opt incompatible with our tile_position ldweights
        return _orig_run_command(cmd, **kw)

    _bu.run_command = _run_command_ldw
    _bu._ldw_opt_patched = True

BF16 = mybir.dt.bfloat16
F32 = mybir.dt.float32

NQ = 32          # query slots
NK = 32          # key slots
D = 256          # input dim (contraction of projections)
A = 256          # attn dim (contraction of logits)
O = 256          # out dim
BS = 4096
N_CORES = 8
BS_CORE = BS // N_CORES   # 512


def build_kernel(bs_core=BS_CORE, n_halves=2):
    """Builds the per-core Bass graph. bs_core must be divisible by 16*n_halves."""
    nc = bacc.Bacc()

    b_h = bs_core // n_halves          # batch per half (256)
    n_groups = b_h // 4                # 4-batch groups per half (64)
    n_gpairs = n_groups // 2

    qT = nc.declare_dram_parameter("qT", [NQ, D, bs_core], BF16, isOutput=False)
    kT = nc.declare_dram_parameter("kT", [NK, D, bs_core], BF16, isOutput=False)
    # merged per-slot weights: [slot, d, 3 (q/k/v), a]
    wall = nc.declare_dram_parameter("wall", [NQ, D, 3, A], BF16,
                                     isOutput=False)
    out = nc.declare_dram_parameter("out", [NQ, bs_core, O], F32, isOutput=True)

    SG = 2  # slots per input DMA group
    # [slot, d, b] -> partition = d%128, chunk c = d//128
    qT_g = qT.rearrange("(sg s) (c p) b -> sg p (s c) b", p=128, s=SG)
    kT_g = kT.rearrange("(sg s) (c p) b -> sg p (s c) b", p=128, s=SG)
    wall_g = wall.rearrange("(sg s) (c p) w a -> sg p (s c) (w a)", p=128, s=SG)

    with tile.TileContext(nc) as tc:
        with (
            tc.tile_pool(name="const", bufs=1) as const_pool,
            tc.tile_pool(name="win", bufs=2) as win,
            tc.tile_pool(name="xin", bufs=4) as xin,
            tc.tile_pool(name="big", bufs=1) as big,
            tc.tile_pool(name="outp", bufs=2) as outp,
            tc.tile_pool(name="vpool", bufs=1) as vpool,
            tc.tile_pool(name="etp", bufs=6) as etp,
            tc.tile_pool(name="proj_ps", bufs=2, space="PSUM") as proj_ps,
            tc.tile_pool(name="lg_ps", bufs=2, space="PSUM") as lg_ps,
            tc.tile_pool(name="av_ps", bufs=4, space="PSUM") as av_ps,
        ):
            identity = const_pool.tile([128, 128], BF16)
            make_identity(nc, identity)

            for half in range(n_halves):
                b0 = half * b_h
                # ---- Phase A: projections -> QTs/KTs (a,b) + VN (b,o) ----
                QTs = big.tile([128, 2, NQ, b_h], BF16, tag="QTs")
                KTs = big.tile([128, 2, NK, b_h], BF16, tag="KTs")
                # value in [b, o] layout: partition = b%128, bc = b//128;
                # four sub-slabs (by m-range) so the V32Q shuffle can start
                # before the whole projection phase finishes
                n_bc = (b_h + 127) // 128
                bw = min(128, b_h)
                VN0 = big.tile([128, n_bc, NK // 4, O], BF16, tag="VN0")
                VN1 = big.tile([128, n_bc, NK // 4, O], BF16, tag="VN1")
                VN2 = big.tile([128, n_bc, NK // 4, O], BF16, tag="VN2")
                VN3 = big.tile([128, n_bc, NK // 4, O], BF16, tag="VN3")
                VNs = (VN0, VN1, VN2, VN3)

                for sg in range(NQ // SG):
                    qts = xin.tile([128, SG, 2, b_h], BF16, tag="qts")
                    nc.sync.dma_start(out=qts,
                                      in_=qT_g[sg, :, :, b0:b0 + b_h])
                    kts = xin.tile([128, SG, 2, b_h], BF16, tag="kts")
                    nc.sync.dma_start(out=kts,
                                      in_=kT_g[sg, :, :, b0:b0 + b_h])
                    wsg = win.tile([128, SG, 2, 3, A], BF16, tag="wsg")
                    nc.sync.dma_start(out=wsg, in_=wall_g[sg])

                    for si in range(SG):
                        s = sg * SG + si
                        # Q and K projections: out = [a, b] per slot
                        for pi in range(2):
                            xs = qts if pi == 0 else kts
                            dst = QTs if pi == 0 else KTs
                            for t in range(2):  # a-tile
                                ps = proj_ps.tile([128, b_h], F32, tag="ps")
                                for c in range(2):
                                    nc.tensor.matmul(
                                        ps,
                                        lhsT=wsg[:, si, c, pi,
                                                 t * 128:(t + 1) * 128],
                                        rhs=xs[:, si, c, :],
                                        start=(c == 0),
                                        stop=(c == 1),
                                    )
                                if pi == 0:
                                    if t == 0:
                                        nc.scalar.mul(dst[:, t, s, :], ps,
                                                      1.0 / 16.0)
                                    else:
                                        nc.vector.tensor_scalar_mul(
                                            out=dst[:, t, s, :], in0=ps,
                                            scalar1=1.0 / 16.0)
                                else:
                                    if t == 0:
                                        nc.scalar.copy(out=dst[:, t, s, :],
                                                       in_=ps)
                                    else:
                                        nc.vector.tensor_copy(
                                            out=dst[:, t, s, :], in_=ps)
                        # V projection transposed: stationary = k chunk,
                        # moving = vw -> psum [b_chunk, o]
                        for bc in range(n_bc):  # b-chunk of 128
                            ps = proj_ps.tile([128, O], F32, tag="ps")
                            for c in range(2):
                                nc.tensor.matmul(
                                    ps[:bw, :],
                                    lhsT=kts[:, si, c,
                                             bc * bw:(bc + 1) * bw],
                                    rhs=wsg[:, si, c, 2, :],
                                    start=(c == 0),
                                    stop=(c == 1),
                                )
                            if bc == 0:
                                nc.scalar.copy(
                                    out=VNs[s // 8][:bw, bc, s % 8, :],
                                    in_=ps[:bw, :])
                            else:
                                nc.vector.tensor_copy(
                                    out=VNs[s // 8][:bw, bc, s % 8, :],
                                    in_=ps[:bw, :])

                # ---- V shuffle: V32Q[32r+m, g, o] = value[b0+64r+g][m, o]
                V32Q = vpool.tile([128, n_groups, O], BF16, tag="V32Q")
                _dmae = (nc.gpsimd, nc.sync)
                di = 0
                for m in range(NK):
                    for r in range(4):
                        row = 32 * r + m
                        b_lo = (r * n_groups) % 128
                        _dmae[di % 2].dma_start(
                            out=V32Q[row:row + 1, :, :],
                            in_=VNs[m // 8][b_lo:b_lo + n_groups,
                                            (r * n_groups) // 128, m % 8, :],
                        )
                        di += 1

                # ---- Phase B1: logits + exp + rowsum, batched by group-quad ----
                rs = big.tile([128, n_groups], F32, tag="rs")
                E = big.tile([128, n_groups, NK], BF16, tag="E")

                n_quads = n_groups // 4
                for gq in range(n_quads):
                    lg = lg_ps.tile([128, 4, NK], F32, tag="lg")
                    for qi in range(4):
                        g = 4 * gq + qi
                        for c in range(2):  # waves: 4 col groups concurrent
                            for j in range(4):
                                b = g + n_groups * j
                                nc.tensor.matmul(
                                    lg[32 * j:32 * (j + 1), qi, :],
                                    lhsT=QTs[:, c, :, b],
                                    rhs=KTs[:, c, :, b],
                                    start=(c == 0),
                                    stop=(c == 1),
                                    tile_position=(0, 32 * j),
                                    skip_group_check=True,
                                )
                    # softmax over m without max-subtraction: logits carry
                    # the 1/16 so |logit| <= ~2 and exp cannot overflow;
                    # normalization is folded into the output copy
                    sm = big.tile([128, 4], F32, tag="sm")
                    nc.scalar.activation(
                        out=E[:, 4 * gq:4 * gq + 4, :], in_=lg,
                        func=mybir.ActivationFunctionType.Exp,
                    )
                    nc.vector.reduce_sum(
                        out=sm, in_=E[:, 4 * gq:4 * gq + 4, :],
                        axis=mybir.AxisListType.X,
                    )
                    nc.vector.reciprocal(out=rs[:, 4 * gq:4 * gq + 4], in_=sm)

                g_chunk = min(8, n_groups)
                for g0 in range(0, n_groups, g_chunk):
                    OUTo = outp.tile([128, g_chunk, O], F32, tag="OUTo")
                    for g in range(g0, g0 + g_chunk):
                        # DVE 32x32 block transpose: TE[32j+m, n] =
                        # E[32j+n, g, m] = batch (g+64j)'s attn transposed
                        te = etp.tile([128, NK], BF16, tag="te")
                        nc.vector.transpose(out=te, in_=E[:, g, :])
                        av = av_ps.tile([128, O], F32, tag="av")
                        for j in range(4):
                            nc.tensor.matmul(
                                av[32 * j:32 * (j + 1), :],
                                lhsT=te[32 * j:32 * (j + 1), :],
                                rhs=V32Q[32 * j:32 * (j + 1), g, :],
                                start=True, stop=True,
                                tile_position=(32 * j, 32 * j),
                                skip_group_check=True,
                            )
                        # psum -> sbuf with 1/softmax-sum scaling per row
                        if g % 2 == 0:
                            nc.scalar.mul(OUTo[:, g - g0, :], av,
                                          rs[:, g:g + 1])
                        else:
                            nc.vector.tensor_scalar_mul(
                                out=OUTo[:, g - g0, :], in0=av,
                                scalar1=rs[:, g:g + 1])
                    # flush: 4 DMAs, one per batch stripe j (batches
                    # b0 + 64j + g0 .. +g_chunk are consecutive)
                    for j in range(4):
                        nc.sync.dma_start(
                            out=out[:, b0 + n_groups * j + g0:
                                    b0 + n_groups * j + g0 + g_chunk, :],
                            in_=OUTo[32 * j:32 * (j + 1), :, :],
                        )
    return nc


def _prep_inputs(q, k, query_weight, key_weight, value_weight, bs_core):
    bf = ml_dtypes.bfloat16
    wall = np.ascontiguousarray(
        np.stack((query_weight, key_weight, value_weight), axis=2)
    ).astype(bf)  # [slot, d, 3, a]
    in_maps = []
    for i in range(N_CORES):
        sl = slice(i * bs_core, (i + 1) * bs_core)
        qTb = np.ascontiguousarray(q[:, sl, :].transpose(0, 2, 1)).astype(bf)
        kTb = np.ascontiguousarray(k[:, sl, :].transpose(0, 2, 1)).astype(bf)
        in_maps.append({"qT": qTb, "kT": kTb, "wall": wall})
    return in_maps


_NC_CACHE = {}


def _get_nc(bs_core, n_halves=2):
    key = (bs_core, n_halves)
    if key not in _NC_CACHE:
        nc = build_kernel(bs_core, n_halves)
        nc.finalize()
        _NC_CACHE[key] = nc
    return _NC_CACHE[key]


def kernel(q, k, query_weight, key_weight, value_weight, _trace=False):
    nc = _get_nc(BS_CORE)
    in_maps = _prep_inputs(q, k, query_weight, key_weight, value_weight, BS_CORE)
    res = run_bass_kernel_spmd(nc, in_maps, core_ids=list(range(N_CORES)),
                               trace=_trace)
    outs = [res.results[i]["out"] for i in range(N_CORES)]
    full = np.concatenate(outs, axis=1).astype(np.float32)
    if _trace:
        return full, res
    return full
